# revision 23
# baseline (speedup 1.0000x reference)
"""Trainium2 Bass kernel for nn_CoarseSkeletonHead (MHA joint decoder).

Data-parallel over B_T across 8 NeuronCores; each core runs 128 batch
elements of: MHA(joint queries, z) -> regressor -> offsets -> ancestor
accumulation.

Host-side algebraic folds (batch-independent weight prep):
  - q = joint_queries @ wq.T + bq is constant  =>  fold wk into it:
    scores = Qt @ z^T + beta,  Qt[(h,j),c] = (q_h @ wk_h)/8,
    beta[(h,j)] = (q_h . bk_h)/8.  Removes the K projection entirely.
  - softmax rows sum to 1  =>  attn @ (zWv + bv) = attn @ zWv + bv,
    so bv folds into out_proj bias: bo' = bo + bv @ Wo.T.
  - SiLU(x) = 0.5 x (1 + tanh(x/2)); the 0.5 folds into w2.
    (tanh shares the ACT table set with exp -> only 2 table loads total.)
  - ancestor matmul handled as block-diag(A^T x5) [120,120] stationary
    weight applied to 120-token (5-batch) offset tiles.
"""

import contextlib
import sys

import numpy as np

sys.path.insert(0, "/opt/trn_rl_repo")

import concourse.bass as bass  # noqa: E402
import concourse.tile as tile  # noqa: E402
from concourse import bacc, mybir  # noqa: E402

F32 = mybir.dt.float32
BF16 = mybir.dt.bfloat16
AF = mybir.ActivationFunctionType
ALU = mybir.AluOpType
AX = mybir.AxisListType

J = 24          # joints
E = 512
H = 8
HD = 64
S = 128
B_TOTAL = 1024
N_CORES = 8
B_CORE = B_TOTAL // N_CORES

PARENT = [-1, 0, 0, 0, 1, 2, 3, 4, 5, 6, 7, 8, 9, 9, 9, 12, 13, 14, 16, 17,
          18, 19, 20, 21]


def _ancestor_matrix():
    A = np.eye(J, dtype=np.float64)
    for j in range(1, J):
        A[j] += A[PARENT[j]]
    return A.astype(np.float32)


def host_prep(joint_queries, in_proj_w, in_proj_b, out_proj_w, out_proj_b,
              w1, b1, ln_g, ln_b, w2, b2):
    jq = np.asarray(joint_queries, np.float32).reshape(J, E)
    ipw = np.asarray(in_proj_w, np.float32)
    ipb = np.asarray(in_proj_b, np.float32)
    wq, wk, wv = ipw[:E], ipw[E:2 * E], ipw[2 * E:]
    bq, bk, bv = ipb[:E], ipb[E:2 * E], ipb[2 * E:]
    wo = np.asarray(out_proj_w, np.float32)
    bo = np.asarray(out_proj_b, np.float32)

    q = jq @ wq.T + bq                                  # [24, 512]
    sc = np.float32(1.0 / np.sqrt(HD))
    qt_rows = np.zeros((H * J, E), np.float32)          # [(h,j), c]
    beta = np.zeros((H * J,), np.float32)
    for h in range(H):
        qh = q[:, h * HD:(h + 1) * HD]
        qt_rows[h * J:(h + 1) * J] = (qh @ wk[h * HD:(h + 1) * HD]) * sc
        beta[h * J:(h + 1) * J] = (qh @ bk[h * HD:(h + 1) * HD]) * sc

    def chunked_T(w):
        # [E_out, E_in] -> w.T in sbuf chunk layout [128, 4*E_out]
        wt = np.ascontiguousarray(w.T.astype(np.float32))
        return np.ascontiguousarray(
            wt.reshape(4, 128, wt.shape[1]).transpose(1, 0, 2).reshape(128, -1))

    A5 = np.zeros((120, 120), np.float32)
    At = _ancestor_matrix().T
    for i in range(5):
        A5[i * J:(i + 1) * J, i * J:(i + 1) * J] = At

    maskj = np.ones((120, 1), np.float32)
    maskj[::J] = 0.0

    consts = {
        "qt": chunked_T(qt_rows),                       # [128, 768]
        "beta": np.ascontiguousarray(beta.reshape(2, 96).T),   # [96, 2]
        "wvt": chunked_T(wv),                           # [128, 2048]
        "wot": chunked_T(wo),                           # [128, 2048]
        "w1t": chunked_T(np.asarray(w1, np.float32)),   # [128, 2048]
        "w2t": chunked_T(np.asarray(w2, np.float32) * 0.5),  # [128, 16]
        "bo4": np.ascontiguousarray((bo + bv @ wo.T).reshape(4, 128).T),
        "b1r": np.broadcast_to(np.asarray(b1, np.float32), (128, E)).copy(),
        "gr": np.broadcast_to(np.asarray(ln_g, np.float32), (128, E)).copy(),
        "br": np.broadcast_to(np.asarray(ln_b, np.float32), (128, E)).copy(),
        "b2r": np.broadcast_to(np.asarray(b2, np.float32), (128, 4)).copy(),
        "a5t": A5,                                      # [120, 120]
        "ident": np.eye(128, dtype=np.float32),
        "maskj": maskj,                                 # [120, 1]
    }
    return consts


CONST_SPECS = {
    "qt": ([128, 768], BF16), "beta": ([96, 2], F32),
    "wvt": ([128, 2048], BF16), "wot": ([128, 2048], BF16),
    "w1t": ([128, 2048], BF16), "w2t": ([128, 16], BF16),
    "bo4": ([128, 4], F32), "b1r": ([128, E], F32), "gr": ([128, E], F32),
    "br": ([128, E], F32), "b2r": ([128, 4], F32), "a5t": ([120, 120], F32),
    "ident": ([128, 128], F32), "maskj": ([120, 1], F32),
}


def _newton_rsqrt(nc, pool, v, tw, out=None, n_iter=2, tag="nr"):
    """y ~= 1/sqrt(v) elementwise on a small [tw, w] fp32 tile, DVE only.

    Quake-style seed via bitcast + 2 Newton iterations (~1e-6 rel).
    If `out` is given, the final iteration writes there.
    """
    w = v.shape[-1]
    yb = pool.tile([v.shape[0], w], F32, tag=tag + "_y")
    t1 = pool.tile([v.shape[0], w], F32, tag=tag + "_t")
    I32 = mybir.dt.int32
    # y_bits = 0x5f3759df - (v_bits >> 1)
    nc.vector.tensor_scalar(
        yb[:tw].bitcast(I32), v[:tw].bitcast(I32),
        scalar1=1, scalar2=None, op0=ALU.logical_shift_right)
    nc.vector.tensor_scalar(
        yb[:tw].bitcast(I32), yb[:tw].bitcast(I32),
        scalar1=-1, scalar2=0x5F3759DF, op0=ALU.mult, op1=ALU.add)
    for i in range(n_iter):
        # y = y * (1.5 - 0.5 v y^2)
        nc.vector.scalar_tensor_tensor(
            t1[:tw], yb[:tw], 1.0, yb[:tw], op0=ALU.bypass, op1=ALU.mult)
        nc.vector.scalar_tensor_tensor(
            t1[:tw], v[:tw], -0.5, t1[:tw], op0=ALU.mult, op1=ALU.mult)
        nc.vector.tensor_scalar_add(t1[:tw], t1[:tw], 1.5)
        dst = yb if (i < n_iter - 1 or out is None) else out
        nc.vector.scalar_tensor_tensor(
            dst[:tw], yb[:tw], 1.0, t1[:tw], op0=ALU.bypass, op1=ALU.mult)
    return out if out is not None else yb


def build_bass(b_core=B_CORE, G=8):
    assert b_core % G == 0 and G % 4 == 0
    NGRP = b_core // G
    T = b_core * J
    ST = 384
    NST = (T + ST - 1) // ST
    TT = 120
    n_tt = (T + TT - 1) // TT

    nc = bacc.Bacc("TRN2", target_bir_lowering=False, debug=False)

    z = nc.dram_tensor("z", [b_core, S, E], F32, kind="ExternalInput").ap()
    dconst = {k: nc.dram_tensor(k, list(shape), F32, kind="ExternalInput").ap()
              for k, (shape, _dt) in CONST_SPECS.items()}

    o_joints = nc.dram_tensor("joints", [b_core, J, 3], F32,
                              kind="ExternalOutput").ap()
    o_offsets = nc.dram_tensor("offsets", [b_core, J, 3], F32,
                               kind="ExternalOutput").ap()
    o_length = nc.dram_tensor("length", [b_core, J - 1], F32,
                              kind="ExternalOutput").ap()

    with tile.TileContext(nc) as tc, contextlib.ExitStack() as ctx:
        cpool = ctx.enter_context(tc.tile_pool(name="consts", bufs=1))
        cs = {}
        for k, (shape, dt) in CONST_SPECS.items():
            t = cpool.tile(shape, dt, tag=k)
            if dt == F32:
                nc.sync.dma_start(t[:], dconst[k])
            else:
                nc.gpsimd.dma_start(t[:], dconst[k])   # SWDGE casts to bf16
            cs[k] = t
        qt, wvt, wot, w1t, w2t = cs["qt"], cs["wvt"], cs["wot"], cs["w1t"], cs["w2t"]
        identb = cpool.tile([128, 128], BF16, tag="identb")
        nc.gpsimd.dma_start(identb[:], dconst["ident"])
        identf = cs["ident"]

        big = ctx.enter_context(tc.tile_pool(name="big", bufs=1))
        ot_all = big.tile([128, 4 * T], BF16, tag="ot_all")   # [(pair,d), (ck, t)]
        jft = big.tile([128, 4 * T], BF16, tag="jft")         # [e', (ck', t)]
        len_row = big.tile([1, T], F32, tag="len_row")
        offs_all = big.tile([120, 3 * n_tt], F32, tag="offs_all")
        joints_all = big.tile([120, 3 * n_tt], F32, tag="joints_all")
        f_all = big.tile([120, 4 * n_tt], F32, tag="f_all")
        rno_all = big.tile([120, n_tt], F32, tag="rno_all")
        nc.gpsimd.memset(f_all[:], 0.0)
        nc.gpsimd.memset(rno_all[:], 0.0)

        # ================= phase A: attention =================
        pa = ctx.enter_context(tc.tile_pool(name="pa", bufs=2))
        with tc.tile_pool(name="ps_sc", bufs=2, space="PSUM") as ps_sc, \
             tc.tile_pool(name="ps_tp", bufs=2, space="PSUM") as ps_tp, \
             tc.tile_pool(name="ps_v", bufs=2, space="PSUM") as ps_v, \
             tc.tile_pool(name="ps_ot", bufs=1, space="PSUM") as ps_ot:
            for g in range(NGRP):
                b0 = g * G
                zg = pa.tile([128, G * E], BF16, tag="zg")
                nc.gpsimd.dma_start(
                    zg[:].rearrange("p (b e) -> p b e", b=G),
                    z[b0:b0 + G].rearrange("b s e -> s b e"))

                # z^T chunks: [c, (ck, b, s)]
                zt = pa.tile([128, 4 * G * S], BF16, tag="zt")
                for ck in range(4):
                    for bq in range(G // 4):
                        pzt = ps_tp.tile([128, 512], BF16, tag="ptp")
                        for i in range(4):
                            b = bq * 4 + i
                            nc.tensor.transpose(
                                pzt[:, i * 128:(i + 1) * 128],
                                zg[:, b * E + ck * 128: b * E + (ck + 1) * 128],
                                identb[:])
                        nc.any.tensor_copy(
                            zt[:, (ck * G + bq * 4) * S:(ck * G + bq * 4 + 4) * S],
                            pzt[:])

                # scores = Qt @ z^T (+beta) -> exp -> attn [96, (half, b, s)]
                attn = pa.tile([96, 2 * G * S], BF16, tag="attn")
                for half in range(2):
                    for bq in range(G // 4):
                        sc = ps_sc.tile([96, 512], F32, tag="sc")
                        for ck in range(4):
                            nc.tensor.matmul(
                                sc[:],
                                qt[:, ck * 192 + half * 96: ck * 192 + half * 96 + 96],
                                zt[:, (ck * G + bq * 4) * S:(ck * G + bq * 4 + 4) * S],
                                start=(ck == 0), stop=(ck == 3))
                        nc.scalar.activation(
                            attn[:, (half * G + bq * 4) * S:(half * G + bq * 4 + 4) * S],
                            sc[:], AF.Exp, bias=cs["beta"][:, half:half + 1])

                # softmax denominators + normalize
                sums = pa.tile([96, 2 * G], F32, tag="sums")
                nc.vector.tensor_reduce(
                    sums[:], attn[:].rearrange("p (k s) -> p k s", s=S),
                    axis=AX.X, op=ALU.add)
                rsum = pa.tile([96, 2 * G], F32, tag="rsum")
                nc.vector.reciprocal(rsum[:], sums[:])
                attn_n = pa.tile([96, 2 * G * S], BF16, tag="attn_n")
                for seg in range(2 * G):
                    nc.vector.tensor_scalar_mul(
                        attn_n[:, seg * S:(seg + 1) * S],
                        attn[:, seg * S:(seg + 1) * S],
                        rsum[:, seg:seg + 1])

                # attn^T: [s, (b, h, j)]
                att = pa.tile([128, G * 192], BF16, tag="att")
                for b in range(G):
                    pat = ps_tp.tile([128, 192], BF16, tag="ptp")
                    for half in range(2):
                        nc.tensor.transpose(
                            pat[:, half * 96:(half + 1) * 96],
                            attn_n[:, (half * G + b) * S:(half * G + b + 1) * S],
                            identb[:96, :96])
                    nc.any.tensor_copy(att[:, b * 192:(b + 1) * 192], pat[:])

                # V = z @ wv^T (bf16 in sbuf)
                vsb = pa.tile([128, G * E], BF16, tag="vsb")
                for b in range(G):
                    pv = ps_v.tile([128, 512], F32, tag="pv")
                    for ck in range(4):
                        nc.tensor.matmul(
                            pv[:],
                            zt[:, (ck * G + b) * S:(ck * G + b + 1) * S],
                            wvt[:, ck * 512:(ck + 1) * 512],
                            start=(ck == 0), stop=(ck == 3))
                    nc.any.tensor_copy(vsb[:, b * E:(b + 1) * E], pv[:])

                # o^T blocks [(h%2)*64 +: 64, j] per (h, b)
                for bq in range(G // 4):
                    pot = [ps_ot.tile([128, 192], F32, tag=f"pot{i}",
                                      name=f"pot{i}") for i in range(2)]
                    for i in range(4):
                        b = bq * 4 + i
                        for h in range(H):
                            p = h // 2
                            dst = pot[p // 2]
                            cb = (p % 2) * 96 + i * 24
                            nc.tensor.matmul(
                                dst[(h % 2) * 64:(h % 2) * 64 + 64, cb:cb + 24],
                                vsb[:, b * E + h * 64: b * E + h * 64 + 64],
                                att[:, b * 192 + h * 24: b * 192 + h * 24 + 24],
                                start=True, stop=True,
                                tile_position=(0, (h % 2) * 64))
                    for p in range(4):
                        nc.any.tensor_copy(
                            ot_all[:, p * T + (b0 + bq * 4) * 24:
                                   p * T + (b0 + bq * 4 + 4) * 24],
                            pot[p // 2][:, (p % 2) * 96:(p % 2) * 96 + 96])

        # ================= phase B: regressor =================
        pb = ctx.enter_context(tc.tile_pool(name="pb", bufs=2))
        with tc.tile_pool(name="ps_jf", bufs=2, space="PSUM") as ps_jf, \
             tc.tile_pool(name="ps_h", bufs=2, space="PSUM") as ps_h, \
             tc.tile_pool(name="ps_w2", bufs=3, space="PSUM") as ps_w2:
            ps_j = ps_w2

            # jf^T = Wo^T . o^T + bo'
            for st in range(NST):
                t0 = st * ST
                tw = min(ST, T - t0)
                for ckp in range(4):
                    pjf = ps_jf.tile([128, ST], F32, tag="pjf")
                    for ck in range(4):
                        nc.tensor.matmul(
                            pjf[:, :tw],
                            wot[:, ck * 512 + ckp * 128: ck * 512 + (ckp + 1) * 128],
                            ot_all[:, ck * T + t0: ck * T + t0 + tw],
                            start=(ck == 0), stop=(ck == 3))
                    nc.scalar.activation(
                        jft[:, ckp * T + t0: ckp * T + t0 + tw],
                        pjf[:, :tw], AF.Identity, bias=cs["bo4"][:, ckp:ckp + 1])

            for it in range(n_tt):
                t0 = it * TT
                tw = min(TT, T - t0)
                # h1 = jf @ w1^T  [t, f] in psum
                ph = ps_h.tile([TT, 512], F32, tag="ph")
                for ckp in range(4):
                    nc.tensor.matmul(
                        ph[:tw, :],
                        jft[:, ckp * T + t0: ckp * T + t0 + tw],
                        w1t[:, ckp * 512:(ckp + 1) * 512],
                        start=(ckp == 0), stop=(ckp == 3))
                # h = ph + b1 (+ row sums);  hsq = h^2 (+ row sums)
                h = pb.tile([TT, 512], F32, tag="h")
                husum = pb.tile([TT, 1], F32, tag="husum")
                nc.vector.scalar_tensor_tensor(
                    h[:tw], ph[:tw], 0.0, cs["b1r"][:tw], op0=ALU.add,
                    op1=ALU.add, accum_out=husum[:tw])
                hsq = pb.tile([TT, 512], F32, tag="hsq")
                hsqsum = pb.tile([TT, 1], F32, tag="hsqsum")
                nc.vector.scalar_tensor_tensor(
                    hsq[:tw], h[:tw], 1.0, h[:tw], op0=ALU.bypass,
                    op1=ALU.mult, accum_out=hsqsum[:tw])
                # mu, var, istd
                mu = pb.tile([TT, 1], F32, tag="mu")
                nc.vector.tensor_scalar_mul(mu[:tw], husum[:tw], 1.0 / E)
                mu2 = pb.tile([TT, 1], F32, tag="mu2")
                nc.vector.tensor_scalar(
                    mu2[:tw], mu[:tw], scalar1=mu[:tw], scalar2=None,
                    op0=ALU.mult)
                var = pb.tile([TT, 1], F32, tag="var")
                nc.vector.scalar_tensor_tensor(
                    var[:tw], hsqsum[:tw], 1.0 / E, mu2[:tw],
                    op0=ALU.mult, op1=ALU.subtract)
                nc.vector.tensor_scalar_add(var[:tw], var[:tw], 1e-5)
                istd = _newton_rsqrt(nc, pb, var, tw, tag="istd")
                # xn = (h - mu) * istd ; y_pre = xn*g + b
                muistd = pb.tile([TT, 1], F32, tag="muistd")
                nc.vector.tensor_scalar(
                    muistd[:tw], mu[:tw], scalar1=istd[:tw], scalar2=None,
                    op0=ALU.mult)
                xn = pb.tile([TT, 512], F32, tag="xn")
                nc.vector.tensor_scalar(
                    xn[:tw], h[:tw], scalar1=istd[:tw], scalar2=muistd[:tw],
                    op0=ALU.mult, op1=ALU.subtract)
                xg = pb.tile([TT, 512], F32, tag="xg")
                nc.any.tensor_tensor(xg[:tw], xn[:tw], cs["gr"][:tw],
                                     op=ALU.mult)
                xb = pb.tile([TT, 512], F32, tag="xb")
                nc.any.tensor_tensor(xb[:tw], xg[:tw], cs["br"][:tw],
                                     op=ALU.add)
                # SiLU via tanh: y = xb * (1 + tanh(xb/2));  (0.5 in w2)
                th = pb.tile([TT, 512], F32, tag="th")
                nc.scalar.activation(th[:tw], xb[:tw], AF.Tanh, scale=0.5)
                nc.vector.tensor_scalar_add(th[:tw], th[:tw], 1.0)
                sy = pb.tile([TT, 512], BF16, tag="sy")
                nc.any.tensor_tensor(sy[:tw], xb[:tw], th[:tw], op=ALU.mult)

                # h^T (bf16) then raw = 2*(h*0.5) @ w2^T + b2
                pht = ps_h.tile([128, 4 * TT], BF16, tag="ph")
                for ck in range(4):
                    nc.tensor.transpose(
                        pht[:, ck * TT: ck * TT + tw],
                        sy[:tw, ck * 128:(ck + 1) * 128],
                        identb[:tw, :tw])
                ht = pb.tile([128, 4 * TT], BF16, tag="ht")
                nc.any.tensor_copy(
                    ht[:].rearrange("p (k t) -> p k t", t=TT)[:, :, :tw],
                    pht[:].rearrange("p (k t) -> p k t", t=TT)[:, :, :tw])
                praw = ps_w2.tile([TT, 4], F32, tag="small")
                for ck in range(4):
                    nc.tensor.matmul(
                        praw[:tw], ht[:, ck * TT: ck * TT + tw],
                        w2t[:, ck * 4:(ck + 1) * 4],
                        start=(ck == 0), stop=(ck == 3))

                # raw + b2 into the wide staging tile; direction rnorm
                nc.vector.tensor_tensor(
                    f_all[:tw, it * 4:(it + 1) * 4], praw[:tw],
                    cs["b2r"][:tw], op=ALU.add)
                f3 = f_all[:, it * 4:it * 4 + 3]
                sq3 = pb.tile([TT, 3], F32, tag="sq3")
                nc.vector.scalar_tensor_tensor(
                    sq3[:tw], f3[:tw], 1.0, f3[:tw],
                    op0=ALU.bypass, op1=ALU.mult)
                nsq = pb.tile([TT, 1], F32, tag="nsq")
                nc.vector.tensor_reduce(nsq[:tw], sq3[:tw], axis=AX.X,
                                        op=ALU.add)
                _newton_rsqrt(nc, pb, nsq, tw, out=rno_all[:, it:it + 1],
                              tag="rno")

            # ---- final pass: softplus(len) = ln(1+exp), offsets, joints ----
            nc.vector.tensor_scalar_min(rno_all[:], rno_all[:], 1e6)
            lfe = pb.tile([120, n_tt], F32, tag="lfe")
            nc.scalar.activation(
                lfe[:], f_all[:].rearrange("p (k f) -> p k f", f=4)[:, :, 3:4],
                AF.Exp)
            nc.vector.tensor_scalar_add(lfe[:], lfe[:], 1.0)
            lfl = pb.tile([120, n_tt], F32, tag="lfl")
            nc.scalar.activation(lfl[:], lfe[:], AF.Ln)
            # masked length * rnorm -> per-token offset scale
            scl_all = pb.tile([120, n_tt], F32, tag="scl_all")
            nc.vector.tensor_scalar_mul(scl_all[:], lfl[:], cs["maskj"][:])
            nc.vector.tensor_tensor(scl_all[:], scl_all[:], rno_all[:],
                                    op=ALU.mult)

            for it in range(n_tt):
                t0 = it * TT
                tw = min(TT, T - t0)
                nc.vector.tensor_scalar_mul(
                    offs_all[:tw, it * 3:(it + 1) * 3],
                    f_all[:tw, it * 4:it * 4 + 3], scl_all[:tw, it:it + 1])

                plen = ps_w2.tile([1, TT], F32, tag="small", name="plen")
                nc.tensor.transpose(plen[:, :tw], lfl[:tw, it:it + 1],
                                    identf[:tw, :tw])
                nc.any.tensor_copy(len_row[:, t0:t0 + tw], plen[:, :tw])

                pj = ps_j.tile([120, 3], F32, tag="small", name="pj")
                nc.tensor.matmul(pj[:tw], cs["a5t"][:tw, :tw],
                                 offs_all[:tw, it * 3:(it + 1) * 3],
                                 start=True, stop=True)
                nc.any.tensor_copy(joints_all[:tw, it * 3:(it + 1) * 3],
                                   pj[:tw])

                nb = tw // J
                bb = t0 // J
                nc.sync.dma_start(
                    o_offsets[bb:bb + nb].rearrange("b j d -> (b j) d"),
                    offs_all[:tw, it * 3:(it + 1) * 3])
                nc.sync.dma_start(
                    o_joints[bb:bb + nb].rearrange("b j d -> (b j) d"),
                    joints_all[:tw, it * 3:(it + 1) * 3])

            nc.sync.dma_start(
                o_length,
                len_row[:, :].rearrange("p (b j) -> p b j", j=J)[:, :, 1:J])

    nc.compile()
    return nc


# ----------------------------------------------------------------------
_BUILT = {}


def _get_built(b_core=B_CORE, G=8):
    key = (b_core, G)
    if key not in _BUILT:
        _BUILT[key] = build_bass(b_core, G)
    return _BUILT[key]


LAST_RESULTS = None


def kernel(z_sequence, joint_queries, in_proj_w, in_proj_b, out_proj_w,
           out_proj_b, w1, b1, ln_g, ln_b, w2, b2, parent):
    global LAST_RESULTS
    import os

    from concourse.bass_utils import run_bass_kernel_spmd

    zs = np.asarray(z_sequence, np.float32)
    consts = host_prep(joint_queries, in_proj_w, in_proj_b, out_proj_w,
                       out_proj_b, w1, b1, ln_g, ln_b, w2, b2)

    nc = _get_built()
    in_maps = []
    for c in range(N_CORES):
        m = dict(consts)
        m["z"] = np.ascontiguousarray(zs[c * B_CORE:(c + 1) * B_CORE])
        in_maps.append(m)

    trace = os.environ.get("KERNEL_TRACE", "0") == "1"
    res = run_bass_kernel_spmd(nc, in_maps, core_ids=list(range(N_CORES)),
                               trace=trace)
    LAST_RESULTS = res
    joints = np.concatenate([r["joints"] for r in res.results], axis=0)
    offsets = np.concatenate([r["offsets"] for r in res.results], axis=0)
    length = np.concatenate([r["length"] for r in res.results], axis=0)
    return joints, offsets, length


# revision 25
# speedup vs baseline: 155.4420x; 155.4420x over previous
"""Trainium2 Bass kernel for nn_CoarseSkeletonHead (MHA joint decoder).

Data-parallel over B_T across 8 NeuronCores; each core runs 128 batch
elements of: MHA(joint queries, z) -> regressor -> offsets -> ancestor
accumulation.

Host-side algebraic folds (batch-independent weight prep):
  - q = joint_queries @ wq.T + bq is constant  =>  fold wk into it:
    scores = Qt @ z^T + beta,  Qt[(h,j),c] = (q_h @ wk_h)/8,
    beta[(h,j)] = (q_h . bk_h)/8.  Removes the K projection entirely.
  - softmax rows sum to 1  =>  attn @ (zWv + bv) = attn @ zWv + bv,
    so bv folds into out_proj bias: bo' = bo + bv @ Wo.T.
  - SiLU(x) = 0.5 x (1 + tanh(x/2)); the 0.5 folds into w2.
    (tanh shares the ACT table set with exp -> only 2 table loads total.)
  - ancestor matmul handled as block-diag(A^T x5) [120,120] stationary
    weight applied to 120-token (5-batch) offset tiles.
"""

import contextlib
import sys

import numpy as np

sys.path.insert(0, "/opt/trn_rl_repo")

import concourse.bass as bass  # noqa: E402
import concourse.tile as tile  # noqa: E402
from concourse import bacc, mybir  # noqa: E402

F32 = mybir.dt.float32
BF16 = mybir.dt.bfloat16
AF = mybir.ActivationFunctionType
ALU = mybir.AluOpType
AX = mybir.AxisListType

J = 24          # joints
E = 512
H = 8
HD = 64
S = 128
B_TOTAL = 1024
N_CORES = 8
B_CORE = B_TOTAL // N_CORES

PARENT = [-1, 0, 0, 0, 1, 2, 3, 4, 5, 6, 7, 8, 9, 9, 9, 12, 13, 14, 16, 17,
          18, 19, 20, 21]


def _ancestor_matrix():
    A = np.eye(J, dtype=np.float64)
    for j in range(1, J):
        A[j] += A[PARENT[j]]
    return A.astype(np.float32)


def host_prep(joint_queries, in_proj_w, in_proj_b, out_proj_w, out_proj_b,
              w1, b1, ln_g, ln_b, w2, b2):
    jq = np.asarray(joint_queries, np.float32).reshape(J, E)
    ipw = np.asarray(in_proj_w, np.float32)
    ipb = np.asarray(in_proj_b, np.float32)
    wq, wk, wv = ipw[:E], ipw[E:2 * E], ipw[2 * E:]
    bq, bk, bv = ipb[:E], ipb[E:2 * E], ipb[2 * E:]
    wo = np.asarray(out_proj_w, np.float32)
    bo = np.asarray(out_proj_b, np.float32)

    q = jq @ wq.T + bq                                  # [24, 512]
    sc = np.float32(1.0 / np.sqrt(HD))
    qt_rows = np.zeros((H * J, E), np.float32)          # [(h,j), c]
    beta = np.zeros((H * J,), np.float32)
    for h in range(H):
        qh = q[:, h * HD:(h + 1) * HD]
        qt_rows[h * J:(h + 1) * J] = (qh @ wk[h * HD:(h + 1) * HD]) * sc
        beta[h * J:(h + 1) * J] = (qh @ bk[h * HD:(h + 1) * HD]) * sc

    def chunked_T(w):
        # [E_out, E_in] -> w.T in sbuf chunk layout [128, 4*E_out]
        wt = np.ascontiguousarray(w.T.astype(np.float32))
        return np.ascontiguousarray(
            wt.reshape(4, 128, wt.shape[1]).transpose(1, 0, 2).reshape(128, -1))

    A5 = np.zeros((120, 120), np.float32)
    At = _ancestor_matrix().T
    for i in range(5):
        A5[i * J:(i + 1) * J, i * J:(i + 1) * J] = At

    maskj = np.ones((120, 1), np.float32)
    maskj[::J] = 0.0

    consts = {
        "qt": chunked_T(qt_rows),                       # [128, 768]
        "beta": np.ascontiguousarray(beta.reshape(2, 96).T),   # [96, 2]
        "wvt": chunked_T(wv),                           # [128, 2048]
        "wot": chunked_T(wo),                           # [128, 2048]
        "w1t": chunked_T(np.asarray(w1, np.float32)),   # [128, 2048]
        "w2t": chunked_T(np.asarray(w2, np.float32) * 0.5),  # [128, 16]
        "bo4": np.ascontiguousarray((bo + bv @ wo.T).reshape(4, 128).T),
        "b1r": np.broadcast_to(np.asarray(b1, np.float32), (128, E)).copy(),
        "gr": np.broadcast_to(np.asarray(ln_g, np.float32), (128, E)).copy(),
        "br": np.broadcast_to(np.asarray(ln_b, np.float32), (128, E)).copy(),
        "b2r": np.broadcast_to(np.asarray(b2, np.float32), (128, 4)).copy(),
        "a5t": A5,                                      # [120, 120]
        "ident": np.eye(128, dtype=np.float32),
        "maskj": maskj,                                 # [120, 1]
    }
    return consts


CONST_SPECS = {
    "qt": ([128, 768], BF16), "beta": ([96, 2], F32),
    "wvt": ([128, 2048], BF16), "wot": ([128, 2048], BF16),
    "w1t": ([128, 2048], BF16), "w2t": ([128, 16], BF16),
    "bo4": ([128, 4], F32), "b1r": ([128, E], F32), "gr": ([128, E], F32),
    "br": ([128, E], F32), "b2r": ([128, 4], F32), "a5t": ([120, 120], F32),
    "ident": ([128, 128], F32), "maskj": ([120, 1], F32),
}


def _newton_rsqrt(nc, pool, v, tw, out=None, n_iter=2, tag="nr"):
    """y ~= 1/sqrt(v) elementwise on a small [tw, w] fp32 tile, DVE only.

    Quake-style seed via bitcast + 2 Newton iterations (~1e-6 rel).
    If `out` is given, the final iteration writes there.
    """
    w = v.shape[-1]
    yb = pool.tile([v.shape[0], w], F32, tag=tag + "_y")
    t1 = pool.tile([v.shape[0], w], F32, tag=tag + "_t")
    I32 = mybir.dt.int32
    # y_bits = 0x5f3759df - (v_bits >> 1)
    nc.vector.tensor_scalar(
        yb[:tw].bitcast(I32), v[:tw].bitcast(I32),
        scalar1=1, scalar2=None, op0=ALU.logical_shift_right)
    nc.vector.tensor_scalar(
        yb[:tw].bitcast(I32), yb[:tw].bitcast(I32),
        scalar1=-1, scalar2=0x5F3759DF, op0=ALU.mult, op1=ALU.add)
    for i in range(n_iter):
        # y = y * (1.5 - 0.5 v y^2)
        nc.vector.scalar_tensor_tensor(
            t1[:tw], yb[:tw], 1.0, yb[:tw], op0=ALU.bypass, op1=ALU.mult)
        nc.vector.scalar_tensor_tensor(
            t1[:tw], v[:tw], -0.5, t1[:tw], op0=ALU.mult, op1=ALU.mult)
        nc.vector.tensor_scalar_add(t1[:tw], t1[:tw], 1.5)
        dst = yb if (i < n_iter - 1 or out is None) else out
        nc.vector.scalar_tensor_tensor(
            dst[:tw], yb[:tw], 1.0, t1[:tw], op0=ALU.bypass, op1=ALU.mult)
    return out if out is not None else yb


def build_bass(b_core=B_CORE, G=8):
    assert b_core % G == 0 and G % 4 == 0
    NGRP = b_core // G
    T = b_core * J
    ST = 384
    NST = (T + ST - 1) // ST
    TT = 120
    n_tt = (T + TT - 1) // TT

    nc = bacc.Bacc("TRN2", target_bir_lowering=False, debug=False)

    z = nc.dram_tensor("z", [b_core, S, E], F32, kind="ExternalInput").ap()
    dconst = {k: nc.dram_tensor(k, list(shape), F32, kind="ExternalInput").ap()
              for k, (shape, _dt) in CONST_SPECS.items()}

    o_joints = nc.dram_tensor("joints", [b_core, J, 3], F32,
                              kind="ExternalOutput").ap()
    o_offsets = nc.dram_tensor("offsets", [b_core, J, 3], F32,
                               kind="ExternalOutput").ap()
    o_length = nc.dram_tensor("length", [b_core, J - 1], F32,
                              kind="ExternalOutput").ap()

    with tile.TileContext(nc) as tc, contextlib.ExitStack() as ctx:
        cpool = ctx.enter_context(tc.tile_pool(name="consts", bufs=1))
        cs = {}
        for k, (shape, dt) in CONST_SPECS.items():
            t = cpool.tile(shape, dt, tag=k)
            if dt == F32:
                nc.sync.dma_start(t[:], dconst[k])
            else:
                nc.gpsimd.dma_start(t[:], dconst[k])   # SWDGE casts to bf16
            cs[k] = t
        qt, wvt, wot, w1t, w2t = cs["qt"], cs["wvt"], cs["wot"], cs["w1t"], cs["w2t"]
        identb = cpool.tile([128, 128], BF16, tag="identb")
        nc.gpsimd.dma_start(identb[:], dconst["ident"])
        identf = cs["ident"]

        big = ctx.enter_context(tc.tile_pool(name="big", bufs=1))
        ot_all = big.tile([128, 4 * T], BF16, tag="ot_all")   # [(pair,d), (ck, t)]
        jft = big.tile([128, 4 * T], BF16, tag="jft")         # [e', (ck', t)]
        len_row = big.tile([1, T], F32, tag="len_row")
        offs_all = big.tile([120, 3 * n_tt], F32, tag="offs_all")
        joints_all = big.tile([120, 3 * n_tt], F32, tag="joints_all")
        f_all = big.tile([120, 4 * n_tt], F32, tag="f_all")
        rno_all = big.tile([120, n_tt], F32, tag="rno_all")
        nc.gpsimd.memset(f_all[:], 0.0)
        nc.gpsimd.memset(rno_all[:], 0.0)

        # ================= phase A: attention =================
        pa = ctx.enter_context(tc.tile_pool(name="pa", bufs=2))
        with tc.tile_pool(name="ps_sc", bufs=2, space="PSUM") as ps_sc, \
             tc.tile_pool(name="ps_tp", bufs=2, space="PSUM") as ps_tp, \
             tc.tile_pool(name="ps_v", bufs=2, space="PSUM") as ps_v, \
             tc.tile_pool(name="ps_ot", bufs=1, space="PSUM") as ps_ot:
            for g in range(NGRP):
                b0 = g * G
                zg = pa.tile([128, G * E], BF16, tag="zg")
                nc.gpsimd.dma_start(
                    zg[:].rearrange("p (b e) -> p b e", b=G),
                    z[b0:b0 + G].rearrange("b s e -> s b e"))

                # z^T chunks: [c, (ck, b, s)]
                zt = pa.tile([128, 4 * G * S], BF16, tag="zt")
                for ck in range(4):
                    for bq in range(G // 4):
                        pzt = ps_tp.tile([128, 512], BF16, tag="ptp")
                        for i in range(4):
                            b = bq * 4 + i
                            nc.tensor.transpose(
                                pzt[:, i * 128:(i + 1) * 128],
                                zg[:, b * E + ck * 128: b * E + (ck + 1) * 128],
                                identb[:])
                        nc.any.tensor_copy(
                            zt[:, (ck * G + bq * 4) * S:(ck * G + bq * 4 + 4) * S],
                            pzt[:])

                # scores = Qt @ z^T (+beta) -> exp -> attn [96, (half, b, s)]
                attn = pa.tile([96, 2 * G * S], BF16, tag="attn")
                for half in range(2):
                    for bq in range(G // 4):
                        sc = ps_sc.tile([96, 512], F32, tag="sc")
                        for ck in range(4):
                            nc.tensor.matmul(
                                sc[:],
                                qt[:, ck * 192 + half * 96: ck * 192 + half * 96 + 96],
                                zt[:, (ck * G + bq * 4) * S:(ck * G + bq * 4 + 4) * S],
                                start=(ck == 0), stop=(ck == 3))
                        nc.scalar.activation(
                            attn[:, (half * G + bq * 4) * S:(half * G + bq * 4 + 4) * S],
                            sc[:], AF.Exp, bias=cs["beta"][:, half:half + 1])

                # softmax denominators + normalize
                sums = pa.tile([96, 2 * G], F32, tag="sums")
                nc.vector.tensor_reduce(
                    sums[:], attn[:].rearrange("p (k s) -> p k s", s=S),
                    axis=AX.X, op=ALU.add)
                rsum = pa.tile([96, 2 * G], F32, tag="rsum")
                nc.vector.reciprocal(rsum[:], sums[:])
                attn_n = pa.tile([96, 2 * G * S], BF16, tag="attn_n")
                for seg in range(2 * G):
                    nc.vector.tensor_scalar_mul(
                        attn_n[:, seg * S:(seg + 1) * S],
                        attn[:, seg * S:(seg + 1) * S],
                        rsum[:, seg:seg + 1])

                # attn^T: [s, (b, h, j)]
                att = pa.tile([128, G * 192], BF16, tag="att")
                for b in range(G):
                    pat = ps_tp.tile([128, 192], BF16, tag="ptp")
                    for half in range(2):
                        nc.tensor.transpose(
                            pat[:, half * 96:(half + 1) * 96],
                            attn_n[:, (half * G + b) * S:(half * G + b + 1) * S],
                            identb[:96, :96])
                    nc.any.tensor_copy(att[:, b * 192:(b + 1) * 192], pat[:])

                # V = z @ wv^T (bf16 in sbuf)
                vsb = pa.tile([128, G * E], BF16, tag="vsb")
                for b in range(G):
                    pv = ps_v.tile([128, 512], F32, tag="pv")
                    for ck in range(4):
                        nc.tensor.matmul(
                            pv[:],
                            zt[:, (ck * G + b) * S:(ck * G + b + 1) * S],
                            wvt[:, ck * 512:(ck + 1) * 512],
                            start=(ck == 0), stop=(ck == 3))
                    nc.any.tensor_copy(vsb[:, b * E:(b + 1) * E], pv[:])

                # o^T blocks [(h%2)*64 +: 64, j] per (h, b)
                for bq in range(G // 4):
                    pot = [ps_ot.tile([128, 192], F32, tag=f"pot{i}",
                                      name=f"pot{i}") for i in range(2)]
                    for i in range(4):
                        b = bq * 4 + i
                        for h in range(H):
                            p = h // 2
                            dst = pot[p // 2]
                            cb = (p % 2) * 96 + i * 24
                            nc.tensor.matmul(
                                dst[(h % 2) * 64:(h % 2) * 64 + 64, cb:cb + 24],
                                vsb[:, b * E + h * 64: b * E + h * 64 + 64],
                                att[:, b * 192 + h * 24: b * 192 + h * 24 + 24],
                                start=True, stop=True,
                                tile_position=(0, (h % 2) * 64))
                    for p in range(4):
                        nc.any.tensor_copy(
                            ot_all[:, p * T + (b0 + bq * 4) * 24:
                                   p * T + (b0 + bq * 4 + 4) * 24],
                            pot[p // 2][:, (p % 2) * 96:(p % 2) * 96 + 96])

        # ================= phase B: regressor =================
        pb = ctx.enter_context(tc.tile_pool(name="pb", bufs=2))
        with tc.tile_pool(name="ps_jf", bufs=2, space="PSUM") as ps_jf, \
             tc.tile_pool(name="ps_h", bufs=2, space="PSUM") as ps_h, \
             tc.tile_pool(name="ps_w2", bufs=3, space="PSUM") as ps_w2:
            ps_j = ps_w2

            # jf^T = Wo^T . o^T + bo'
            for st in range(NST):
                t0 = st * ST
                tw = min(ST, T - t0)
                for ckp in range(4):
                    pjf = ps_jf.tile([128, ST], F32, tag="pjf")
                    for ck in range(4):
                        nc.tensor.matmul(
                            pjf[:, :tw],
                            wot[:, ck * 512 + ckp * 128: ck * 512 + (ckp + 1) * 128],
                            ot_all[:, ck * T + t0: ck * T + t0 + tw],
                            start=(ck == 0), stop=(ck == 3))
                    nc.scalar.activation(
                        jft[:, ckp * T + t0: ckp * T + t0 + tw],
                        pjf[:, :tw], AF.Identity, bias=cs["bo4"][:, ckp:ckp + 1])

            for it in range(n_tt):
                t0 = it * TT
                tw = min(TT, T - t0)
                # h1 = jf @ w1^T  [t, f] in psum
                ph = ps_h.tile([TT, 512], F32, tag="ph")
                for ckp in range(4):
                    nc.tensor.matmul(
                        ph[:tw, :],
                        jft[:, ckp * T + t0: ckp * T + t0 + tw],
                        w1t[:, ckp * 512:(ckp + 1) * 512],
                        start=(ckp == 0), stop=(ckp == 3))
                # h = ph + b1 (+ row sums);  hsq = h^2 (+ row sums)
                h = pb.tile([TT, 512], F32, tag="h")
                husum = pb.tile([TT, 1], F32, tag="husum")
                nc.vector.scalar_tensor_tensor(
                    h[:tw], ph[:tw], 0.0, cs["b1r"][:tw], op0=ALU.add,
                    op1=ALU.add, accum_out=husum[:tw])
                hsq = pb.tile([TT, 512], F32, tag="hsq")
                hsqsum = pb.tile([TT, 1], F32, tag="hsqsum")
                nc.vector.scalar_tensor_tensor(
                    hsq[:tw], h[:tw], 1.0, h[:tw], op0=ALU.bypass,
                    op1=ALU.mult, accum_out=hsqsum[:tw])
                # mu, var, istd
                mu = pb.tile([TT, 1], F32, tag="mu")
                nc.vector.tensor_scalar_mul(mu[:tw], husum[:tw], 1.0 / E)
                mu2 = pb.tile([TT, 1], F32, tag="mu2")
                nc.vector.tensor_scalar(
                    mu2[:tw], mu[:tw], scalar1=mu[:tw], scalar2=None,
                    op0=ALU.mult)
                var = pb.tile([TT, 1], F32, tag="var")
                nc.vector.scalar_tensor_tensor(
                    var[:tw], hsqsum[:tw], 1.0 / E, mu2[:tw],
                    op0=ALU.mult, op1=ALU.subtract)
                nc.vector.tensor_scalar_add(var[:tw], var[:tw], 1e-5)
                istd = _newton_rsqrt(nc, pb, var, tw, tag="istd")
                # xn = (h - mu) * istd ; y_pre = xn*g + b
                muistd = pb.tile([TT, 1], F32, tag="muistd")
                nc.vector.tensor_scalar(
                    muistd[:tw], mu[:tw], scalar1=istd[:tw], scalar2=None,
                    op0=ALU.mult)
                xn = pb.tile([TT, 512], F32, tag="xn")
                nc.vector.tensor_scalar(
                    xn[:tw], h[:tw], scalar1=istd[:tw], scalar2=muistd[:tw],
                    op0=ALU.mult, op1=ALU.subtract)
                xg = pb.tile([TT, 512], F32, tag="xg")
                nc.any.tensor_tensor(xg[:tw], xn[:tw], cs["gr"][:tw],
                                     op=ALU.mult)
                xb = pb.tile([TT, 512], F32, tag="xb")
                nc.any.tensor_tensor(xb[:tw], xg[:tw], cs["br"][:tw],
                                     op=ALU.add)
                # SiLU via tanh: y = xb * (1 + tanh(xb/2));  (0.5 in w2)
                th = pb.tile([TT, 512], F32, tag="th")
                nc.scalar.activation(th[:tw], xb[:tw], AF.Tanh, scale=0.5)
                nc.vector.tensor_scalar_add(th[:tw], th[:tw], 1.0)
                sy = pb.tile([TT, 512], BF16, tag="sy")
                nc.any.tensor_tensor(sy[:tw], xb[:tw], th[:tw], op=ALU.mult)

                # h^T (bf16) then raw = 2*(h*0.5) @ w2^T + b2
                pht = ps_h.tile([128, 4 * TT], BF16, tag="ph")
                for ck in range(4):
                    nc.tensor.transpose(
                        pht[:, ck * TT: ck * TT + tw],
                        sy[:tw, ck * 128:(ck + 1) * 128],
                        identb[:tw, :tw])
                ht = pb.tile([128, 4 * TT], BF16, tag="ht")
                nc.any.tensor_copy(
                    ht[:].rearrange("p (k t) -> p k t", t=TT)[:, :, :tw],
                    pht[:].rearrange("p (k t) -> p k t", t=TT)[:, :, :tw])
                praw = ps_w2.tile([TT, 4], F32, tag="small")
                for ck in range(4):
                    nc.tensor.matmul(
                        praw[:tw], ht[:, ck * TT: ck * TT + tw],
                        w2t[:, ck * 4:(ck + 1) * 4],
                        start=(ck == 0), stop=(ck == 3))

                # raw + b2 into the wide staging tile; direction rnorm
                nc.vector.tensor_tensor(
                    f_all[:tw, it * 4:(it + 1) * 4], praw[:tw],
                    cs["b2r"][:tw], op=ALU.add)
                f3 = f_all[:, it * 4:it * 4 + 3]
                sq3 = pb.tile([TT, 3], F32, tag="sq3")
                nc.vector.scalar_tensor_tensor(
                    sq3[:tw], f3[:tw], 1.0, f3[:tw],
                    op0=ALU.bypass, op1=ALU.mult)
                nsq = pb.tile([TT, 1], F32, tag="nsq")
                nc.vector.tensor_reduce(nsq[:tw], sq3[:tw], axis=AX.X,
                                        op=ALU.add)
                _newton_rsqrt(nc, pb, nsq, tw, out=rno_all[:, it:it + 1],
                              tag="rno")

            # ---- final pass: softplus(len) = ln(1+exp), offsets, joints ----
            nc.vector.tensor_scalar_min(rno_all[:], rno_all[:], 1e6)
            lfe = pb.tile([120, n_tt], F32, tag="lfe")
            nc.scalar.activation(
                lfe[:], f_all[:].rearrange("p (k f) -> p k f", f=4)[:, :, 3:4],
                AF.Exp)
            nc.vector.tensor_scalar_add(lfe[:], lfe[:], 1.0)
            lfl = pb.tile([120, n_tt], F32, tag="lfl")
            nc.scalar.activation(lfl[:], lfe[:], AF.Ln)
            # masked length * rnorm -> per-token offset scale
            scl_all = pb.tile([120, n_tt], F32, tag="scl_all")
            nc.vector.tensor_scalar_mul(scl_all[:], lfl[:], cs["maskj"][:])
            nc.vector.tensor_tensor(scl_all[:], scl_all[:], rno_all[:],
                                    op=ALU.mult)

            for it in range(n_tt):
                t0 = it * TT
                tw = min(TT, T - t0)
                nc.vector.tensor_scalar_mul(
                    offs_all[:tw, it * 3:(it + 1) * 3],
                    f_all[:tw, it * 4:it * 4 + 3], scl_all[:tw, it:it + 1])

                plen = ps_w2.tile([1, TT], F32, tag="small", name="plen")
                nc.tensor.transpose(plen[:, :tw], lfl[:tw, it:it + 1],
                                    identf[:tw, :tw])
                nc.any.tensor_copy(len_row[:, t0:t0 + tw], plen[:, :tw])

                pj = ps_j.tile([120, 3], F32, tag="small", name="pj")
                nc.tensor.matmul(pj[:tw], cs["a5t"][:tw, :tw],
                                 offs_all[:tw, it * 3:(it + 1) * 3],
                                 start=True, stop=True)
                nc.any.tensor_copy(joints_all[:tw, it * 3:(it + 1) * 3],
                                   pj[:tw])

                nb = tw // J
                bb = t0 // J
                nc.sync.dma_start(
                    o_offsets[bb:bb + nb].rearrange("b j d -> (b j) d"),
                    offs_all[:tw, it * 3:(it + 1) * 3])
                nc.sync.dma_start(
                    o_joints[bb:bb + nb].rearrange("b j d -> (b j) d"),
                    joints_all[:tw, it * 3:(it + 1) * 3])

            nc.sync.dma_start(
                o_length,
                len_row[:, :].rearrange("p (b j) -> p b j", j=J)[:, :, 1:J])

    nc.compile()
    return nc


# ----------------------------------------------------------------------
_BUILT = {}


def _get_built(b_core=B_CORE, G=8):
    key = (b_core, G)
    if key not in _BUILT:
        _BUILT[key] = build_bass(b_core, G)
    return _BUILT[key]


LAST_RESULTS = None


def _install_ntff_shim():
    """Provide antenv.axon_hooks so run_bass_kernel_spmd(trace=True) can
    capture NTFF profiles through libaxon_pjrt.so (missing in this image)."""
    import contextlib
    import ctypes
    import types

    if "antenv.axon_hooks" in sys.modules:
        return
    so_path = "/opt/axon/libaxon_pjrt.so"
    try:
        lib = ctypes.CDLL(so_path)
    except OSError:
        return
    if not hasattr(lib, "axon_start_nrt_profile"):
        return
    lib.axon_start_nrt_profile.argtypes = [
        ctypes.POINTER(ctypes.c_int64), ctypes.c_size_t]
    lib.axon_start_nrt_profile.restype = ctypes.c_int64
    lib.axon_stop_nrt_profile.argtypes = [ctypes.c_char_p]
    lib.axon_stop_nrt_profile.restype = ctypes.c_int64

    @contextlib.contextmanager
    def _hook(output_dir, device_ids):
        import jax
        jax.devices()
        if device_ids:
            ids = (ctypes.c_int64 * len(device_ids))(*device_ids)
            rc = lib.axon_start_nrt_profile(ids, len(device_ids))
        else:
            rc = lib.axon_start_nrt_profile(None, 0)
        if rc != 0:
            raise RuntimeError(f"axon_start_nrt_profile rc={rc}")
        try:
            yield
        finally:
            n = lib.axon_stop_nrt_profile(str(output_dir).encode())
            print(f"ntff profile: {n} file(s) -> {output_dir}")

    mod = types.ModuleType("antenv.axon_hooks")
    mod.get_axon_ntff_profile_hook = lambda: _hook
    mod.set_axon_ntff_profile_hook = lambda h: None
    sys.modules["antenv.axon_hooks"] = mod


def kernel(z_sequence, joint_queries, in_proj_w, in_proj_b, out_proj_w,
           out_proj_b, w1, b1, ln_g, ln_b, w2, b2, parent):
    global LAST_RESULTS
    import os

    from concourse.bass_utils import run_bass_kernel_spmd

    zs = np.asarray(z_sequence, np.float32)
    consts = host_prep(joint_queries, in_proj_w, in_proj_b, out_proj_w,
                       out_proj_b, w1, b1, ln_g, ln_b, w2, b2)

    nc = _get_built()
    in_maps = []
    for c in range(N_CORES):
        m = dict(consts)
        m["z"] = np.ascontiguousarray(zs[c * B_CORE:(c + 1) * B_CORE])
        in_maps.append(m)

    trace = os.environ.get("KERNEL_TRACE", "0") == "1"
    if trace:
        _install_ntff_shim()
    res = run_bass_kernel_spmd(nc, in_maps, core_ids=list(range(N_CORES)),
                               trace=trace,
                               tmpdir=os.environ.get("KERNEL_TRACE_DIR"))
    LAST_RESULTS = res
    joints = np.concatenate([r["joints"] for r in res.results], axis=0)
    offsets = np.concatenate([r["offsets"] for r in res.results], axis=0)
    length = np.concatenate([r["length"] for r in res.results], axis=0)
    return joints, offsets, length


# revision 43
# speedup vs baseline: 156.0651x; 1.0040x over previous
"""Trainium2 Bass kernel for nn_CoarseSkeletonHead (MHA joint decoder).

Data-parallel over B_T across 8 NeuronCores; each core runs 128 batch
elements of: MHA(joint queries, z) -> regressor -> offsets -> ancestor
accumulation.

Host-side algebraic folds (batch-independent weight prep):
  - q = joint_queries @ wq.T + bq is constant  =>  fold wk into it:
    scores = Qt @ z^T + beta,  Qt[(h,j),c] = (q_h @ wk_h)/8,
    beta[(h,j)] = (q_h . bk_h)/8.  Removes the K projection entirely.
  - softmax rows sum to 1  =>  attn @ (zWv + bv) = attn @ zWv + bv,
    so bv folds into out_proj bias: bo' = bo + bv @ Wo.T.
  - SiLU(x) = 0.5 x (1 + tanh(x/2)); the 0.5 folds into w2.
    (tanh shares the ACT table set with exp -> only 2 table loads total.)
  - ancestor matmul handled as block-diag(A^T x5) [120,120] stationary
    weight applied to 120-token (5-batch) offset tiles.
"""

import contextlib
import sys

import numpy as np

sys.path.insert(0, "/opt/trn_rl_repo")

import concourse.bass as bass  # noqa: E402
import concourse.tile as tile  # noqa: E402
from concourse import bacc, mybir  # noqa: E402

F32 = mybir.dt.float32
BF16 = mybir.dt.bfloat16
AF = mybir.ActivationFunctionType
ALU = mybir.AluOpType
AX = mybir.AxisListType

J = 24          # joints
E = 512
H = 8
HD = 64
S = 128
B_TOTAL = 1024
N_CORES = 8
B_CORE = B_TOTAL // N_CORES

PARENT = [-1, 0, 0, 0, 1, 2, 3, 4, 5, 6, 7, 8, 9, 9, 9, 12, 13, 14, 16, 17,
          18, 19, 20, 21]


def _ancestor_matrix():
    A = np.eye(J, dtype=np.float64)
    for j in range(1, J):
        A[j] += A[PARENT[j]]
    return A.astype(np.float32)


def host_prep(joint_queries, in_proj_w, in_proj_b, out_proj_w, out_proj_b,
              w1, b1, ln_g, ln_b, w2, b2):
    jq = np.asarray(joint_queries, np.float32).reshape(J, E)
    ipw = np.asarray(in_proj_w, np.float32)
    ipb = np.asarray(in_proj_b, np.float32)
    wq, wk, wv = ipw[:E], ipw[E:2 * E], ipw[2 * E:]
    bq, bk, bv = ipb[:E], ipb[E:2 * E], ipb[2 * E:]
    wo = np.asarray(out_proj_w, np.float32)
    bo = np.asarray(out_proj_b, np.float32)

    q = jq @ wq.T + bq                                  # [24, 512]
    sc = np.float32(1.0 / np.sqrt(HD))
    qt_rows = np.zeros((H * J, E), np.float32)          # [(h,j), c]
    beta = np.zeros((H * J,), np.float32)
    for h in range(H):
        qh = q[:, h * HD:(h + 1) * HD]
        qt_rows[h * J:(h + 1) * J] = (qh @ wk[h * HD:(h + 1) * HD]) * sc
        beta[h * J:(h + 1) * J] = (qh @ bk[h * HD:(h + 1) * HD]) * sc

    def chunked_T(w):
        # [E_out, E_in] -> w.T in sbuf chunk layout [128, 4*E_out]
        wt = np.ascontiguousarray(w.T.astype(np.float32))
        return np.ascontiguousarray(
            wt.reshape(4, 128, wt.shape[1]).transpose(1, 0, 2).reshape(128, -1))

    A5 = np.zeros((120, 120), np.float32)
    At = _ancestor_matrix().T
    for i in range(5):
        A5[i * J:(i + 1) * J, i * J:(i + 1) * J] = At

    maskj = np.ones((120, 1), np.float32)
    maskj[::J] = 0.0

    consts = {
        "qt": chunked_T(qt_rows),                       # [128, 768]
        "beta": np.ascontiguousarray(beta.reshape(2, 96).T),   # [96, 2]
        "wvt": chunked_T(wv),                           # [128, 2048]
        "wot": chunked_T(wo),                           # [128, 2048]
        "w1t": chunked_T(np.asarray(w1, np.float32)),   # [128, 2048]
        "w2t": chunked_T(np.asarray(w2, np.float32) * 0.5),  # [128, 16]
        "bo4": np.ascontiguousarray((bo + bv @ wo.T).reshape(4, 128).T),
        "b1r": np.broadcast_to(np.asarray(b1, np.float32), (128, E)).copy(),
        "gr": np.broadcast_to(np.asarray(ln_g, np.float32), (128, E)).copy(),
        "br": np.broadcast_to(np.asarray(ln_b, np.float32), (128, E)).copy(),
        "b2r": np.broadcast_to(np.asarray(b2, np.float32), (128, 4)).copy(),
        "a5t": A5,                                      # [120, 120]
        "ident": np.eye(128, dtype=np.float32),
        "maskj": maskj,                                 # [120, 1]
    }
    return consts


CONST_SPECS = {
    "qt": ([128, 768], BF16), "beta": ([96, 2], F32),
    "wvt": ([128, 2048], BF16), "wot": ([128, 2048], BF16),
    "w1t": ([128, 2048], BF16), "w2t": ([128, 16], BF16),
    "bo4": ([128, 4], F32), "b1r": ([128, E], F32), "gr": ([128, E], BF16),
    "br": ([128, E], BF16), "b2r": ([128, 4], F32), "a5t": ([120, 120], F32),
    "ident": ([128, 128], F32), "maskj": ([120, 1], F32),
}


def _newton_rsqrt(nc, pool, v, tw, out=None, n_iter=2, tag="nr"):
    """y ~= 1/sqrt(v) elementwise on a small [tw, w] fp32 tile, DVE only.

    Quake-style seed via bitcast + 2 Newton iterations (~1e-6 rel).
    If `out` is given, the final iteration writes there.
    """
    w = v.shape[-1]
    yb = pool.tile([v.shape[0], w], F32, tag=tag + "_y")
    t1 = pool.tile([v.shape[0], w], F32, tag=tag + "_t")
    I32 = mybir.dt.int32
    # y_bits = 0x5f3759df - (v_bits >> 1)
    nc.vector.tensor_scalar(
        yb[:tw].bitcast(I32), v[:tw].bitcast(I32),
        scalar1=1, scalar2=None, op0=ALU.logical_shift_right)
    nc.vector.tensor_scalar(
        yb[:tw].bitcast(I32), yb[:tw].bitcast(I32),
        scalar1=-1, scalar2=0x5F3759DF, op0=ALU.mult, op1=ALU.add)
    for i in range(n_iter):
        # y = y * (1.5 - 0.5 v y^2)
        nc.vector.scalar_tensor_tensor(
            t1[:tw], yb[:tw], 1.0, yb[:tw], op0=ALU.bypass, op1=ALU.mult)
        nc.vector.scalar_tensor_tensor(
            t1[:tw], v[:tw], -0.5, t1[:tw], op0=ALU.mult, op1=ALU.mult)
        nc.vector.tensor_scalar_add(t1[:tw], t1[:tw], 1.5)
        dst = yb if (i < n_iter - 1 or out is None) else out
        nc.vector.scalar_tensor_tensor(
            dst[:tw], yb[:tw], 1.0, t1[:tw], op0=ALU.bypass, op1=ALU.mult)
    return out if out is not None else yb


def build_bass(b_core=B_CORE, G=8):
    assert b_core % G == 0 and G % 4 == 0
    NGRP = b_core // G
    T = b_core * J
    ST = 384
    NST = (T + ST - 1) // ST
    TT = 120
    n_tt = (T + TT - 1) // TT

    nc = bacc.Bacc("TRN2", target_bir_lowering=False, debug=False)

    z = nc.dram_tensor("z", [b_core, S, E], F32, kind="ExternalInput").ap()
    dconst = {k: nc.dram_tensor(k, list(shape), F32, kind="ExternalInput").ap()
              for k, (shape, _dt) in CONST_SPECS.items()}

    o_joints = nc.dram_tensor("joints", [b_core, J, 3], F32,
                              kind="ExternalOutput").ap()
    o_offsets = nc.dram_tensor("offsets", [b_core, J, 3], F32,
                               kind="ExternalOutput").ap()
    o_length = nc.dram_tensor("length", [b_core, J - 1], F32,
                              kind="ExternalOutput").ap()

    with tile.TileContext(nc) as tc, contextlib.ExitStack() as ctx:
        cpool = ctx.enter_context(tc.tile_pool(name="consts", bufs=1))
        cs = {}
        for k, (shape, dt) in CONST_SPECS.items():
            t = cpool.tile(shape, dt, tag=k)
            if dt == F32:
                nc.sync.dma_start(t[:], dconst[k])
            else:
                nc.gpsimd.dma_start(t[:], dconst[k])   # SWDGE casts to bf16
            cs[k] = t
        qt, wvt, wot, w1t, w2t = cs["qt"], cs["wvt"], cs["wot"], cs["w1t"], cs["w2t"]
        identb = cpool.tile([128, 128], BF16, tag="identb")
        nc.gpsimd.dma_start(identb[:], dconst["ident"])
        identf = cs["ident"]

        big = ctx.enter_context(tc.tile_pool(name="big", bufs=1))
        ot_all = big.tile([128, 4 * T], BF16, tag="ot_all")   # [(pair,d), (ck, t)]
        jft = big.tile([128, 4 * T], BF16, tag="jft")         # [e', (ck', t)]
        len_row = big.tile([1, T], F32, tag="len_row")
        offs_all = big.tile([120, 3 * n_tt], F32, tag="offs_all")
        joints_all = big.tile([120, 3 * n_tt], F32, tag="joints_all")
        f_all = big.tile([120, 4 * n_tt], F32, tag="f_all")
        rno_all = big.tile([120, n_tt], F32, tag="rno_all")
        nc.gpsimd.memset(f_all[:], 0.0)
        nc.gpsimd.memset(rno_all[:], 0.0)

        # ================= phase A: attention =================
        pa = ctx.enter_context(tc.tile_pool(name="pa", bufs=2))
        ps_sc = ctx.enter_context(tc.tile_pool(name="ps_sc", bufs=2, space="PSUM"))
        ps_tp = ctx.enter_context(tc.tile_pool(name="ps_tp", bufs=2, space="PSUM"))
        ps_v = ctx.enter_context(tc.tile_pool(name="ps_v", bufs=2, space="PSUM"))
        ps_ot = ctx.enter_context(tc.tile_pool(name="ps_ot", bufs=2, space="PSUM"))
        if True:
            for g in range(NGRP):
                b0 = g * G
                zg = pa.tile([128, G * E], BF16, tag="zg")
                nc.gpsimd.dma_start(
                    zg[:].rearrange("p (b e) -> p b e", b=G),
                    z[b0:b0 + G].rearrange("b s e -> s b e"))

                # z^T chunks: [c, (ck, b, s)]
                zt = pa.tile([128, 4 * G * S], BF16, tag="zt")
                for ck in range(4):
                    for bq in range(G // 4):
                        pzt = ps_tp.tile([128, 512], BF16, tag="pzt")
                        for i in range(4):
                            b = bq * 4 + i
                            nc.tensor.transpose(
                                pzt[:, i * 128:(i + 1) * 128],
                                zg[:, b * E + ck * 128: b * E + (ck + 1) * 128],
                                identb[:])
                        nc.any.tensor_copy(
                            zt[:, (ck * G + bq * 4) * S:(ck * G + bq * 4 + 4) * S],
                            pzt[:])

                # scores = Qt @ z^T (+beta) -> exp -> attn [96, (half, b, s)]
                attn = pa.tile([96, 2 * G * S], BF16, tag="attn")
                for half in range(2):
                    for bq in range(G // 4):
                        sc = ps_sc.tile([96, 512], F32, tag="sc")
                        for ck in range(4):
                            nc.tensor.matmul(
                                sc[:],
                                qt[:, ck * 192 + half * 96: ck * 192 + half * 96 + 96],
                                zt[:, (ck * G + bq * 4) * S:(ck * G + bq * 4 + 4) * S],
                                start=(ck == 0), stop=(ck == 3))
                        nc.scalar.activation(
                            attn[:, (half * G + bq * 4) * S:(half * G + bq * 4 + 4) * S],
                            sc[:], AF.Exp, bias=cs["beta"][:, half:half + 1])

                # softmax denominators + normalize
                sums = pa.tile([96, 2 * G], F32, tag="sums")
                nc.vector.tensor_reduce(
                    sums[:], attn[:].rearrange("p (k s) -> p k s", s=S),
                    axis=AX.X, op=ALU.add)
                rsum = pa.tile([96, 2 * G], F32, tag="rsum")
                nc.vector.reciprocal(rsum[:], sums[:])
                attn_n = pa.tile([96, 2 * G * S], BF16, tag="attn_n")
                for seg in range(2 * G):
                    nc.vector.tensor_scalar_mul(
                        attn_n[:, seg * S:(seg + 1) * S],
                        attn[:, seg * S:(seg + 1) * S],
                        rsum[:, seg:seg + 1])

                # attn^T: [s, (b, h, j)]
                att = pa.tile([128, G * 192], BF16, tag="att")
                for b in range(G):
                    pat = ps_tp.tile([128, 192], BF16, tag="pzt", name="pat")
                    for half in range(2):
                        nc.tensor.transpose(
                            pat[:, half * 96:(half + 1) * 96],
                            attn_n[:, (half * G + b) * S:(half * G + b + 1) * S],
                            identb[:96, :96])
                    nc.any.tensor_copy(att[:, b * 192:(b + 1) * 192], pat[:])

                # V = z @ wv^T (independent of softmax; keeps PE fed)
                vsb = pa.tile([128, G * E], BF16, tag="vsb")
                for b in range(G):
                    pv = ps_v.tile([128, 512], F32, tag="pv")
                    for ck in range(4):
                        nc.tensor.matmul(
                            pv[:],
                            zt[:, (ck * G + b) * S:(ck * G + b + 1) * S],
                            wvt[:, ck * 512:(ck + 1) * 512],
                            start=(ck == 0), stop=(ck == 3))
                    nc.any.tensor_copy(vsb[:, b * E:(b + 1) * E], pv[:])

                # o^T blocks [(h%2)*64 +: 64, j] per (h, b)
                for bq in range(G // 4):
                    pot = [ps_ot.tile([128, 192], F32, tag=f"pot{i}",
                                      name=f"pot{i}", bufs=1) for i in range(2)]
                    for i in range(4):
                        b = bq * 4 + i
                        for h in range(H):
                            p = h // 2
                            dst = pot[p // 2]
                            cb = (p % 2) * 96 + i * 24
                            nc.tensor.matmul(
                                dst[(h % 2) * 64:(h % 2) * 64 + 64, cb:cb + 24],
                                vsb[:, b * E + h * 64: b * E + h * 64 + 64],
                                att[:, b * 192 + h * 24: b * 192 + h * 24 + 24],
                                start=True, stop=True,
                                tile_position=(0, (h % 2) * 64))
                    for p in range(4):
                        nc.any.tensor_copy(
                            ot_all[:, p * T + (b0 + bq * 4) * 24:
                                   p * T + (b0 + bq * 4 + 4) * 24],
                            pot[p // 2][:, (p % 2) * 96:(p % 2) * 96 + 96])

        # ================= phase B: regressor =================
        pb = ctx.enter_context(tc.tile_pool(name="pb", bufs=2))
        if True:
            ps_jf, ps_h, ps_w2, ps_j = ps_sc, ps_v, ps_tp, ps_tp

            # jf^T = Wo^T . o^T + bo'
            for st in range(NST):
                t0 = st * ST
                tw = min(ST, T - t0)
                for ckp in range(4):
                    pjf = ps_jf.tile([128, ST], F32, tag="sc", name="pjf")
                    for ck in range(4):
                        nc.tensor.matmul(
                            pjf[:, :tw],
                            wot[:, ck * 512 + ckp * 128: ck * 512 + (ckp + 1) * 128],
                            ot_all[:, ck * T + t0: ck * T + t0 + tw],
                            start=(ck == 0), stop=(ck == 3))
                    nc.scalar.activation(
                        jft[:, ckp * T + t0: ckp * T + t0 + tw],
                        pjf[:, :tw], AF.Identity, bias=cs["bo4"][:, ckp:ckp + 1])

            for it in range(n_tt):
                t0 = it * TT
                tw = min(TT, T - t0)
                # h1 = jf @ w1^T  [t, f] in psum
                ph = ps_h.tile([TT, 512], F32, tag="pv", name="ph")
                for ckp in range(4):
                    nc.tensor.matmul(
                        ph[:tw, :],
                        jft[:, ckp * T + t0: ckp * T + t0 + tw],
                        w1t[:, ckp * 512:(ckp + 1) * 512],
                        start=(ckp == 0), stop=(ckp == 3))
                # h = ph + b1 (+ row sums);  hsq = h^2 (+ row sums)
                h = pb.tile([TT, 512], BF16, tag="h")
                husum = pb.tile([TT, 1], F32, tag="husum")
                nc.vector.scalar_tensor_tensor(
                    h[:tw], ph[:tw], 0.0, cs["b1r"][:tw], op0=ALU.add,
                    op1=ALU.add, accum_out=husum[:tw])
                hsq = pb.tile([TT, 512], BF16, tag="hsq")
                hsqsum = pb.tile([TT, 1], F32, tag="hsqsum")
                nc.vector.scalar_tensor_tensor(
                    hsq[:tw], h[:tw], 1.0, h[:tw], op0=ALU.bypass,
                    op1=ALU.mult, accum_out=hsqsum[:tw])
                # mu, var, istd
                mu = pb.tile([TT, 1], F32, tag="mu")
                nc.vector.tensor_scalar_mul(mu[:tw], husum[:tw], 1.0 / E)
                mu2 = pb.tile([TT, 1], F32, tag="mu2")
                nc.vector.tensor_scalar(
                    mu2[:tw], mu[:tw], scalar1=mu[:tw], scalar2=None,
                    op0=ALU.mult)
                var = pb.tile([TT, 1], F32, tag="var")
                nc.vector.scalar_tensor_tensor(
                    var[:tw], hsqsum[:tw], 1.0 / E, mu2[:tw],
                    op0=ALU.mult, op1=ALU.subtract)
                nc.vector.tensor_scalar_add(var[:tw], var[:tw], 1e-5)
                istd = _newton_rsqrt(nc, pb, var, tw, tag="istd")
                # xn = (h - mu) * istd ; y_pre = xn*g + b
                muistd = pb.tile([TT, 1], F32, tag="muistd")
                nc.vector.tensor_scalar(
                    muistd[:tw], mu[:tw], scalar1=istd[:tw], scalar2=None,
                    op0=ALU.mult)
                xn = pb.tile([TT, 512], BF16, tag="xn")
                nc.vector.tensor_scalar(
                    xn[:tw], h[:tw], scalar1=istd[:tw], scalar2=muistd[:tw],
                    op0=ALU.mult, op1=ALU.subtract)
                xg = pb.tile([TT, 512], BF16, tag="xg")
                nc.any.tensor_tensor(xg[:tw], xn[:tw], cs["gr"][:tw],
                                        op=ALU.mult)
                xb = pb.tile([TT, 512], BF16, tag="xb")
                nc.any.tensor_tensor(xb[:tw], xg[:tw], cs["br"][:tw],
                                        op=ALU.add)
                # SiLU via tanh: y = xb * (1 + tanh(xb/2));  (0.5 in w2)
                th = pb.tile([TT, 512], BF16, tag="th")
                nc.scalar.activation(th[:tw], xb[:tw], AF.Tanh, scale=0.5)
                nc.vector.tensor_scalar_add(th[:tw], th[:tw], 1.0)
                sy = pb.tile([TT, 512], BF16, tag="sy")
                nc.any.tensor_tensor(sy[:tw], xb[:tw], th[:tw], op=ALU.mult)

                # h^T (bf16) then raw = 2*(h*0.5) @ w2^T + b2
                pht = ps_h.tile([128, 4 * TT], BF16, tag="pv", name="pht")
                for ck in range(4):
                    nc.tensor.transpose(
                        pht[:, ck * TT: ck * TT + tw],
                        sy[:tw, ck * 128:(ck + 1) * 128],
                        identb[:tw, :tw])
                ht = pb.tile([128, 4 * TT], BF16, tag="ht")
                nc.scalar.activation(
                    ht[:].rearrange("p (k t) -> p k t", t=TT)[:, :, :tw],
                    pht[:].rearrange("p (k t) -> p k t", t=TT)[:, :, :tw],
                    AF.Copy)
                praw = ps_w2.tile([TT, 4], F32, tag="pzt", name="praw")
                for ck in range(4):
                    nc.tensor.matmul(
                        praw[:tw], ht[:, ck * TT: ck * TT + tw],
                        w2t[:, ck * 4:(ck + 1) * 4],
                        start=(ck == 0), stop=(ck == 3))

                # raw + b2 into the wide staging tile; direction rnorm
                nc.vector.tensor_tensor(
                    f_all[:tw, it * 4:(it + 1) * 4], praw[:tw],
                    cs["b2r"][:tw], op=ALU.add)
                f3 = f_all[:, it * 4:it * 4 + 3]
                sq3 = pb.tile([TT, 3], F32, tag="sq3")
                nc.vector.scalar_tensor_tensor(
                    sq3[:tw], f3[:tw], 1.0, f3[:tw],
                    op0=ALU.bypass, op1=ALU.mult)
                nsq = pb.tile([TT, 1], F32, tag="nsq")
                nc.vector.tensor_reduce(nsq[:tw], sq3[:tw], axis=AX.X,
                                        op=ALU.add)
                _newton_rsqrt(nc, pb, nsq, tw, out=rno_all[:, it:it + 1],
                              tag="rno")

            # ---- final pass: softplus(len) = ln(1+exp), offsets, joints ----
            nc.vector.tensor_scalar_min(rno_all[:], rno_all[:], 1e6)
            lfe = pb.tile([120, n_tt], F32, tag="lfe")
            nc.scalar.activation(
                lfe[:], f_all[:].rearrange("p (k f) -> p k f", f=4)[:, :, 3:4],
                AF.Exp)
            nc.vector.tensor_scalar_add(lfe[:], lfe[:], 1.0)
            lfl = pb.tile([120, n_tt], F32, tag="lfl")
            nc.scalar.activation(lfl[:], lfe[:], AF.Ln)
            # masked length * rnorm -> per-token offset scale
            scl_all = pb.tile([120, n_tt], F32, tag="scl_all")
            nc.vector.tensor_scalar_mul(scl_all[:], lfl[:], cs["maskj"][:])
            nc.vector.tensor_tensor(scl_all[:], scl_all[:], rno_all[:],
                                    op=ALU.mult)

            for it in range(n_tt):
                t0 = it * TT
                tw = min(TT, T - t0)
                nc.vector.tensor_scalar_mul(
                    offs_all[:tw, it * 3:(it + 1) * 3],
                    f_all[:tw, it * 4:it * 4 + 3], scl_all[:tw, it:it + 1])

                plen = ps_w2.tile([1, TT], F32, tag="pzt", name="plen")
                nc.tensor.transpose(plen[:, :tw], lfl[:tw, it:it + 1],
                                    identf[:tw, :tw])
                nc.any.tensor_copy(len_row[:, t0:t0 + tw], plen[:, :tw])

                pj = ps_j.tile([120, 3], F32, tag="pzt", name="pj")
                nc.tensor.matmul(pj[:tw], cs["a5t"][:tw, :tw],
                                 offs_all[:tw, it * 3:(it + 1) * 3],
                                 start=True, stop=True)
                nc.any.tensor_copy(joints_all[:tw, it * 3:(it + 1) * 3],
                                   pj[:tw])

                nb = tw // J
                bb = t0 // J
                nc.sync.dma_start(
                    o_offsets[bb:bb + nb].rearrange("b j d -> (b j) d"),
                    offs_all[:tw, it * 3:(it + 1) * 3])
                nc.sync.dma_start(
                    o_joints[bb:bb + nb].rearrange("b j d -> (b j) d"),
                    joints_all[:tw, it * 3:(it + 1) * 3])

            nc.sync.dma_start(
                o_length,
                len_row[:, :].rearrange("p (b j) -> p b j", j=J)[:, :, 1:J])

    nc.compile()
    return nc


# ----------------------------------------------------------------------
_BUILT = {}


def _get_built(b_core=B_CORE, G=8):
    key = (b_core, G)
    if key not in _BUILT:
        _BUILT[key] = build_bass(b_core, G)
    return _BUILT[key]


LAST_RESULTS = None


def _install_ntff_shim():
    """Provide antenv.axon_hooks so run_bass_kernel_spmd(trace=True) can
    capture NTFF profiles through libaxon_pjrt.so (missing in this image)."""
    import contextlib
    import ctypes
    import types

    if "antenv.axon_hooks" in sys.modules:
        return
    so_path = "/opt/axon/libaxon_pjrt.so"
    try:
        lib = ctypes.CDLL(so_path)
    except OSError:
        return
    if not hasattr(lib, "axon_start_nrt_profile"):
        return
    lib.axon_start_nrt_profile.argtypes = [
        ctypes.POINTER(ctypes.c_int64), ctypes.c_size_t]
    lib.axon_start_nrt_profile.restype = ctypes.c_int64
    lib.axon_stop_nrt_profile.argtypes = [ctypes.c_char_p]
    lib.axon_stop_nrt_profile.restype = ctypes.c_int64

    @contextlib.contextmanager
    def _hook(output_dir, device_ids):
        import jax
        jax.devices()
        if device_ids:
            ids = (ctypes.c_int64 * len(device_ids))(*device_ids)
            rc = lib.axon_start_nrt_profile(ids, len(device_ids))
        else:
            rc = lib.axon_start_nrt_profile(None, 0)
        if rc != 0:
            raise RuntimeError(f"axon_start_nrt_profile rc={rc}")
        try:
            yield
        finally:
            n = lib.axon_stop_nrt_profile(str(output_dir).encode())
            print(f"ntff profile: {n} file(s) -> {output_dir}")

    mod = types.ModuleType("antenv.axon_hooks")
    mod.get_axon_ntff_profile_hook = lambda: _hook
    mod.set_axon_ntff_profile_hook = lambda h: None
    sys.modules["antenv.axon_hooks"] = mod


def kernel(z_sequence, joint_queries, in_proj_w, in_proj_b, out_proj_w,
           out_proj_b, w1, b1, ln_g, ln_b, w2, b2, parent):
    global LAST_RESULTS
    import os

    from concourse.bass_utils import run_bass_kernel_spmd

    zs = np.asarray(z_sequence, np.float32)
    consts = host_prep(joint_queries, in_proj_w, in_proj_b, out_proj_w,
                       out_proj_b, w1, b1, ln_g, ln_b, w2, b2)

    nc = _get_built()
    in_maps = []
    for c in range(N_CORES):
        m = dict(consts)
        m["z"] = np.ascontiguousarray(zs[c * B_CORE:(c + 1) * B_CORE])
        in_maps.append(m)

    trace = os.environ.get("KERNEL_TRACE", "0") == "1"
    if trace:
        _install_ntff_shim()
    res = run_bass_kernel_spmd(nc, in_maps, core_ids=list(range(N_CORES)),
                               trace=trace,
                               tmpdir=os.environ.get("KERNEL_TRACE_DIR"))
    LAST_RESULTS = res
    joints = np.concatenate([r["joints"] for r in res.results], axis=0)
    offsets = np.concatenate([r["offsets"] for r in res.results], axis=0)
    length = np.concatenate([r["length"] for r in res.results], axis=0)
    return joints, offsets, length
